# revision 14
# baseline (speedup 1.0000x reference)
"""Trainium2 Bass kernel for nn_Phaseformer (32 conv branches + degenerate
single-token attention + unfold-mean pool), tensor-parallel over 8 NeuronCores.

Sharding: the 32 conv branches are packed into 16 perfectly balanced
branch-pairs (b, 31-b) whose kernel sizes sum to 33 and output lengths sum to
33; each core owns 2 pairs (= 66 of the 528 concatenated T columns).  Every
core runs the identical SPMD program; all per-branch heterogeneity lives in the
host-prepared input data (weight slabs, im2col operands, masks).

The attention tail is linear in the per-core column slice, so each core
computes a partial of the final (4, 256) output on-device and the host sums
the 8 partials (output-contraction unshard).  No device collective is used.
"""

import os
import numpy as np

import concourse.bass as bass
import concourse.tile as tile
import concourse.mybir as mybir
from concourse.alu_op_type import AluOpType
from concourse.bass_utils import run_bass_kernel_spmd

F32 = mybir.dt.float32
F32R = mybir.dt.float32r
AFT = mybir.ActivationFunctionType

N_CORES = 8
DUR = 32          # duration == number of branches
DIM = 256
T_TOTAL = DUR * (DUR + 1) // 2   # 528
K33 = 33          # taps per branch-pair (k_b + k_b' = 33)
CTRACT = K33 * DIM               # 8448 contraction length per pair GEMM
NCT = CTRACT // 128              # 66 contraction tiles
PAIRS_PER_CORE = 2
W_CHUNK = 6       # contraction tiles per weight DMA (6KB partition lines)
LN_EPS = 1e-5
N_W = 4           # pooled windows
POOL_STEP = 4 * DUR              # 128
SEL_ROWS = 128    # rows of out_proj actually needed (4 windows x 32)
S1_PAD = 640      # 528 padded to 5*128 for the tail matvec

LAST_EXEC_TIME_NS = None
LAST_TRACE_DIR = None

_PROGRAM_CACHE = {}


# --------------------------------------------------------------------------
# axon NTFF profiling hook (used only when tracing is requested)
# --------------------------------------------------------------------------
def _install_ntff_hook():
    import sys, types, ctypes, contextlib
    if 'antenv.axon_hooks' in sys.modules:
        return
    try:
        mod = types.ModuleType('antenv.axon_hooks')
        _state = {}
        mod.set_axon_ntff_profile_hook = lambda h: _state.__setitem__('h', h)
        mod.get_axon_ntff_profile_hook = lambda: _state.get('h')
        sys.modules['antenv.axon_hooks'] = mod
        import antenv
        antenv.axon_hooks = mod

        so_path = '/opt/axon/libaxon_pjrt.so'
        lib = ctypes.CDLL(so_path)
        if not hasattr(lib, 'axon_start_nrt_profile'):
            return
        lib.axon_start_nrt_profile.argtypes = [ctypes.POINTER(ctypes.c_int64),
                                               ctypes.c_size_t]
        lib.axon_start_nrt_profile.restype = ctypes.c_int64
        lib.axon_stop_nrt_profile.argtypes = [ctypes.c_char_p]
        lib.axon_stop_nrt_profile.restype = ctypes.c_int64

        @contextlib.contextmanager
        def _hook(output_dir, device_ids):
            import jax
            jax.devices()
            if device_ids:
                ids = (ctypes.c_int64 * len(device_ids))(*device_ids)
                rc = lib.axon_start_nrt_profile(ids, len(device_ids))
            else:
                rc = lib.axon_start_nrt_profile(None, 0)
            if rc != 0:
                raise RuntimeError(f'axon_start_nrt_profile rc={rc}')
            try:
                yield
            finally:
                n = lib.axon_stop_nrt_profile(str(output_dir).encode())
                print(f'ntff profile: {n} file(s) -> {output_dir}')

        mod.set_axon_ntff_profile_hook(_hook)

        import concourse.bass_utils as bu
        bu.upload_artifacts = lambda tmpdir: f'file://{tmpdir}'
    except Exception as e:  # profiling is best-effort
        print(f'ntff hook install failed: {e}')


# --------------------------------------------------------------------------
# walrus here encodes at most ONE sem wait per instruction; split excess
# waits onto same-engine NoOps inserted just before the instruction.
# --------------------------------------------------------------------------
def _split_excess_waits(nc, max_waits=1):
    for fn in nc.m.functions:
        for bb in fn.blocks:
            new_list = []
            for ins in bb.instructions:
                si = ins.sync_info
                if si is not None and si.on_wait and len(si.on_wait) > max_waits:
                    waits = list(si.on_wait)
                    chunks = [waits[i:i + max_waits]
                              for i in range(0, len(waits), max_waits)]
                    for chunk in chunks[:-1]:
                        nop = mybir.InstNoOp(
                            name=nc.get_next_instruction_name(),
                            engine=ins.engine,
                            sync_info=mybir.SyncInfo(on_wait=list(chunk),
                                                     on_update=[]),
                        )
                        nc.register_instruction(nop)
                        new_list.append(nop)
                    si.on_wait = list(chunks[-1])
                new_list.append(ins)
            bb.instructions[:] = new_list


# --------------------------------------------------------------------------
# pairing / column-map helpers (shapes are structural constants)
# --------------------------------------------------------------------------
def _pair_info(p):
    """Pair p packs branches (b, b') = (p, 31-p): k=b+1 taps, L=32-b cols."""
    b, bp = p, 31 - p
    k, kp = b + 1, bp + 1        # k + kp = 33
    L, Lp = DUR - b, DUR - bp    # L + Lp = 33
    return b, bp, k, kp, L, Lp


def _branch_offset(b):
    # start of branch b inside the reference concat T axis
    return DUR * b - (b * (b - 1)) // 2


# --------------------------------------------------------------------------
# device program (built once, shared by all cores)
# --------------------------------------------------------------------------
def _build_program(w_bufs=4):
    nc = bass.Bass(trn_type="TRN2", target_bir_lowering=False,
                   num_devices=N_CORES)

    wslab = nc.declare_dram_parameter(
        "wslab", [PAIRS_PER_CORE, NCT // W_CHUNK, 128, W_CHUNK * DIM],
        F32, isOutput=False)
    xislab = nc.declare_dram_parameter("xislab", [PAIRS_PER_CORE, 128, NCT * K33],
                                       F32, isOutput=False)
    bias_t = nc.declare_dram_parameter("bias_t", [PAIRS_PER_CORE, K33, DIM],
                                       F32, isOutput=False)
    lnw_t = nc.declare_dram_parameter("lnw_t", [PAIRS_PER_CORE, K33, DIM],
                                      F32, isOutput=False)
    lnb_t = nc.declare_dram_parameter("lnb_t", [PAIRS_PER_CORE, K33, DIM],
                                      F32, isOutput=False)
    segmask = nc.declare_dram_parameter("segmask", [PAIRS_PER_CORE, K33, 2],
                                        F32, isOutput=False)
    segmask_tr = nc.declare_dram_parameter("segmask_tr", [PAIRS_PER_CORE, 2, K33],
                                           F32, isOutput=False)
    ninv = nc.declare_dram_parameter("ninv", [PAIRS_PER_CORE, 2, 1],
                                     F32, isOutput=False)
    wv_cols = nc.declare_dram_parameter("wv_cols", [PAIRS_PER_CORE, K33, S1_PAD],
                                        F32, isOutput=False)
    bv_pad = nc.declare_dram_parameter("bv_pad", [128, S1_PAD // 128], F32,
                                       isOutput=False)
    wout_tr = nc.declare_dram_parameter("wout_tr", [128, S1_PAD], F32,
                                        isOutput=False)
    opb_sel = nc.declare_dram_parameter("opb_sel", [128, 1], F32, isOutput=False)
    winmask = nc.declare_dram_parameter("winmask", [128, N_W], F32,
                                        isOutput=False)
    out = nc.declare_dram_parameter("out", [N_W, DIM], F32, isOutput=True)

    with tile.TileContext(nc) as tc:
        with tc.tile_pool(name="const", bufs=1) as const, \
             tc.tile_pool(name="wpool", bufs=w_bufs) as wpool, \
             tc.tile_pool(name="zpool", bufs=2, space="PSUM") as zpool, \
             tc.tile_pool(name="spsum", bufs=1, space="PSUM") as spsum, \
             tc.tile_pool(name="qpsum", bufs=1, space="PSUM") as qpsum, \
             tc.tile_pool(name="fpsum", bufs=1, space="PSUM") as fpsum, \
             tc.tile_pool(name="work", bufs=2) as work:

            # tiles for per-pair constants; DMAs are issued just-in-time
            # inside the pair loop so the weight stream starts immediately.
            xi_sb, bias_sb, lnw_sb, lnb_sb = [], [], [], []
            segm_sb, segmT_sb, ninv_sb = [], [], []
            for P in range(PAIRS_PER_CORE):
                xi_sb.append(const.tile([128, NCT * K33], F32R,
                                        name=f"xi{P}", tag=f"xi{P}"))
                bias_sb.append(const.tile([K33, DIM], F32,
                                          name=f"bias{P}", tag=f"bias{P}"))
                lnw_sb.append(const.tile([K33, DIM], F32,
                                         name=f"lnw{P}", tag=f"lnw{P}"))
                lnb_sb.append(const.tile([K33, DIM], F32,
                                         name=f"lnb{P}", tag=f"lnb{P}"))
                segm_sb.append(const.tile([K33, 2], F32,
                                          name=f"segm{P}", tag=f"segm{P}"))
                segmT_sb.append(const.tile([2, K33], F32,
                                           name=f"segmT{P}", tag=f"segmT{P}"))
                ninv_sb.append(const.tile([2, 1], F32,
                                          name=f"ninv{P}", tag=f"ninv{P}"))

            u_sb = [const.tile([K33, 1], F32, name=f"u{P}", tag=f"u{P}")
                    for P in range(PAIRS_PER_CORE)]
            wv_sb = []
            bv_sb = const.tile([128, S1_PAD // 128], F32, tag="bv")
            wout_sb = const.tile([128, S1_PAD], F32, tag="wout")
            opb_sb = const.tile([128, 1], F32, tag="opb")
            winm_sb = const.tile([128, N_W], F32, tag="winm")

            # ---- per-pair conv GEMM + fused LN-to-column-sums -----------
            # DMA triggers all run in program order on the SP queue, so the
            # issue order below is the prefetch schedule: first weight chunk
            # and the pair-0 im2col first, then every small constant (they
            # are cheap and needed mid-kernel), then the bulk weight stream.
            NCHUNK = NCT // W_CHUNK
            wt_tiles = [[], []]
            for P in range(PAIRS_PER_CORE):
                for c in range(NCHUNK):
                    wt_tiles[P].append(
                        wpool.tile([128, W_CHUNK * DIM], F32R,
                                   name=f"wt{P}_{c}", tag="w"))

            def dma_chunk(P, c):
                nc.sync.dma_start(wt_tiles[P][c][:], wslab[P, c].bitcast(F32R))

            dma_chunk(0, 0)
            nc.sync.dma_start(xi_sb[0][:], xislab[0].bitcast(F32R))
            dma_chunk(0, 1)
            dma_chunk(0, 2)
            for P in range(PAIRS_PER_CORE):
                nc.sync.dma_start(bias_sb[P][:], bias_t[P])
                nc.sync.dma_start(lnw_sb[P][:], lnw_t[P])
                nc.sync.dma_start(lnb_sb[P][:], lnb_t[P])
                nc.sync.dma_start(segm_sb[P][:], segmask[P])
                nc.sync.dma_start(segmT_sb[P][:], segmask_tr[P])
                nc.sync.dma_start(ninv_sb[P][:], ninv[P])
            nc.sync.dma_start(xi_sb[1][:], xislab[1].bitcast(F32R))
            for P in range(PAIRS_PER_CORE):
                t = const.tile([K33, S1_PAD], F32, name=f"wv{P}", tag=f"wv{P}")
                nc.sync.dma_start(t[:], wv_cols[P])
                wv_sb.append(t)
            nc.sync.dma_start(bv_sb[:], bv_pad[:])
            nc.sync.dma_start(wout_sb[:], wout_tr[:])
            nc.sync.dma_start(opb_sb[:], opb_sel[:])
            nc.sync.dma_start(winm_sb[:], winmask[:])
            for c in range(3, NCHUNK):
                dma_chunk(0, c)
            for c in range(NCHUNK):
                dma_chunk(1, c)

            for P in range(PAIRS_PER_CORE):
                zp = zpool.tile([K33, DIM], F32, tag="z")
                for c in range(NCHUNK):
                    wt = wt_tiles[P][c]
                    for jj in range(W_CHUNK):
                        j = c * W_CHUNK + jj
                        nc.tensor.matmul(
                            zp[:],
                            lhsT=xi_sb[P][:, j * K33:(j + 1) * K33],
                            rhs=wt[:, jj * DIM:(jj + 1) * DIM],
                            start=(j == 0), stop=(j == NCT - 1),
                        )

                # zb = Z^T + bias  (DVE reads PSUM)
                zb = work.tile([K33, DIM], F32, tag="zb")
                nc.vector.scalar_tensor_tensor(
                    out=zb[:], in0=zp[:], scalar=1.0, in1=bias_sb[P][:],
                    op0=AluOpType.mult, op1=AluOpType.add)

                # g = gelu(zb) on ACT engine (exact erf form)
                g = work.tile([K33, DIM], F32, tag="g")
                nc.scalar.activation(g[:], zb[:], AFT.Gelu)

                # per-column (free-dim) reductions -> stk columns
                # 0: sum g, 1: sum g^2, 2: sum g*lnw, 3: sum lnw, 4: sum lnb
                stk = work.tile([K33, 8], F32, tag="stk")
                nc.vector.tensor_reduce(stk[:, 0:1], g[:],
                                        mybir.AxisListType.X, AluOpType.add)
                scr = work.tile([K33, DIM], F32, tag="scr")
                nc.vector.tensor_tensor(scr[:], g[:], g[:], AluOpType.mult)
                nc.vector.tensor_reduce(stk[:, 1:2], scr[:],
                                        mybir.AxisListType.X, AluOpType.add)
                scr2 = work.tile([K33, DIM], F32, tag="scr2")
                nc.vector.tensor_tensor(scr2[:], g[:], lnw_sb[P][:],
                                        AluOpType.mult)
                nc.vector.tensor_reduce(stk[:, 2:3], scr2[:],
                                        mybir.AxisListType.X, AluOpType.add)
                nc.vector.tensor_reduce(stk[:, 3:4], lnw_sb[P][:],
                                        mybir.AxisListType.X, AluOpType.add)
                nc.vector.tensor_reduce(stk[:, 4:5], lnb_sb[P][:],
                                        mybir.AxisListType.X, AluOpType.add)

                # per-branch totals: (2 x 5) = segmask^T @ stk
                bst = spsum.tile([2, 8], F32, tag="bst")
                nc.tensor.matmul(bst[:, 0:5], lhsT=segm_sb[P][:],
                                 rhs=stk[:, 0:5], start=True, stop=True)

                # branch stats -> mu, rstd, rstd*mu   (2-partition vectors)
                st = work.tile([2, 8], F32, tag="st")
                # st0 = mu, st1 = E[y^2], st2 = mu^2, st3 = var
                nc.vector.tensor_tensor(st[:, 0:1], bst[:, 0:1],
                                        ninv_sb[P][:], AluOpType.mult)
                nc.vector.tensor_tensor(st[:, 1:2], bst[:, 1:2],
                                        ninv_sb[P][:], AluOpType.mult)
                nc.vector.tensor_tensor(st[:, 2:3], st[:, 0:1], st[:, 0:1],
                                        AluOpType.mult)
                nc.vector.tensor_tensor(st[:, 3:4], st[:, 1:2], st[:, 2:3],
                                        AluOpType.subtract)
                # st4 = sqrt(var + eps); st5 = 1/st4 = rstd
                nc.vector.tensor_scalar_add(st[:, 3:4], st[:, 3:4], LN_EPS)
                nc.scalar.activation(st[:, 4:5], st[:, 3:4], AFT.Sqrt)
                nc.vector.reciprocal(st[:, 5:6], st[:, 4:5])
                # mr: col0 = rstd, col1 = rstd * mu
                mr = work.tile([2, 2], F32, tag="mr")
                nc.vector.tensor_copy(mr[:, 0:1], st[:, 5:6])
                nc.vector.tensor_tensor(mr[:, 1:2], st[:, 5:6], st[:, 0:1],
                                        AluOpType.mult)

                # broadcast branch scalars to the 33 columns
                bc = spsum.tile([K33, 2], F32, tag="bc")
                nc.tensor.matmul(bc[:], lhsT=segmT_sb[P][:], rhs=mr[:],
                                 start=True, stop=True)

                # u = rstd*cs_glnw - (rstd*mu)*cs_lnw + cs_lnb
                t1 = work.tile([K33, 2], F32, tag="t1")
                nc.vector.tensor_tensor(t1[:, 0:1], stk[:, 2:3], bc[:, 0:1],
                                        AluOpType.mult)
                nc.vector.tensor_tensor(t1[:, 1:2], stk[:, 3:4], bc[:, 1:2],
                                        AluOpType.mult)
                nc.vector.tensor_tensor(t1[:, 0:1], t1[:, 0:1], t1[:, 1:2],
                                        AluOpType.subtract)
                nc.vector.tensor_tensor(u_sb[P][:], t1[:, 0:1], stk[:, 4:5],
                                        AluOpType.add)

            # ---- attention tail (all partial w.r.t. this core) ----------
            # q[128f+p] = sum_cols Wv[128f+p, col] * u[col]; computed directly
            # in partition-major (128, 5) form via transposed matvecs, then
            # + (256/8) * bv.
            NF = S1_PAD // 128
            vq = qpsum.tile([128, NF], F32, tag="vq")
            for f in range(NF):
                for P in range(PAIRS_PER_CORE):
                    nc.tensor.matmul(
                        vq[:, f:f + 1], lhsT=wv_sb[P][:, f * 128:(f + 1) * 128],
                        rhs=u_sb[P][:],
                        start=(P == 0), stop=(P == PAIRS_PER_CORE - 1))
            s1_sb = work.tile([128, NF], F32, tag="s1")
            nc.vector.scalar_tensor_tensor(
                out=s1_sb[:], in0=bv_sb[:], scalar=float(DIM) / N_CORES,
                in1=vq[:], op0=AluOpType.mult, op1=AluOpType.add)

            # v = Wout_sel @ q   (128 selected rows of out_proj)
            vps = fpsum.tile([128, 1], F32, tag="vps")
            for f in range(S1_PAD // 128):
                nc.tensor.matmul(vps[:],
                                 lhsT=wout_sb[:, f * 128:(f + 1) * 128],
                                 rhs=s1_sb[:, f:f + 1],
                                 start=(f == 0), stop=(f == S1_PAD // 128 - 1))

            v2 = work.tile([128, 2], F32, tag="v2")
            nc.vector.tensor_copy(v2[:, 0:1], vps[:])
            nc.vector.tensor_scalar_mul(v2[:, 1:2], opb_sb[:],
                                        float(DIM) / N_CORES)

            # window-mean pooling of the two columns, then add
            ops = fpsum.tile([N_W, 2], F32, tag="ops")
            nc.tensor.matmul(ops[:], lhsT=winm_sb[:], rhs=v2[:],
                             start=True, stop=True)
            o2 = work.tile([N_W, 2], F32, tag="o2")
            nc.vector.tensor_copy(o2[:], ops[:])
            p4 = work.tile([N_W, 1], F32, tag="p4")
            nc.vector.tensor_tensor(p4[:], o2[:, 0:1], o2[:, 1:2],
                                    AluOpType.add)

            # broadcast the 4 window values across the 256 feature dim
            outT = work.tile([N_W, DIM], F32, tag="outT")
            nc.vector.memset(outT[:], 0.0)
            nc.vector.tensor_scalar_add(outT[:], outT[:], p4[:])
            nc.sync.dma_start(out[:], outT[:])

    _split_excess_waits(nc)
    return nc


# --------------------------------------------------------------------------
# host-side sharding (indexing / gather / zero-fill only)
# --------------------------------------------------------------------------
def _host_prepare(inputs):
    x = np.ascontiguousarray(inputs["x"], dtype=np.float32)
    conv_w = np.asarray(inputs["conv_w"], dtype=np.float32)
    conv_b = np.asarray(inputs["conv_b"], dtype=np.float32)
    ln_w = np.asarray(inputs["ln_w"], dtype=np.float32)
    ln_b = np.asarray(inputs["ln_b"], dtype=np.float32)
    in_proj_w = np.asarray(inputs["in_proj_w"], dtype=np.float32)
    in_proj_b = np.asarray(inputs["in_proj_b"], dtype=np.float32)
    out_proj_w = np.asarray(inputs["out_proj_w"], dtype=np.float32)
    out_proj_b = np.asarray(inputs["out_proj_b"], dtype=np.float32)

    xt = np.ascontiguousarray(x[0].T)            # (DIM, DUR)
    Wv = in_proj_w[2 * T_TOTAL:]                 # (T, T) value slice
    bv = in_proj_b[2 * T_TOTAL:]                 # (T,)

    # shared (core-independent) tensors -----------------------------------
    bv_flat = np.zeros(S1_PAD, np.float32)
    bv_flat[:T_TOTAL] = bv
    bv_pad = np.ascontiguousarray(bv_flat.reshape(S1_PAD // 128, 128).T)

    row_sel = np.asarray([POOL_STEP * w + j
                          for w in range(N_W) for j in range(DUR)])
    m = np.zeros((S1_PAD, 128), np.float32)
    m[:T_TOTAL, :] = out_proj_w[row_sel].T       # [s1_idx, sel_row]
    wout_tr = np.ascontiguousarray(
        m.reshape(S1_PAD // 128, 128, 128).transpose(1, 0, 2).reshape(128, S1_PAD))

    opb_sel = np.ascontiguousarray(out_proj_b[row_sel][:, None])
    winmask = np.zeros((128, N_W), np.float32)
    for o in range(128):
        winmask[o, o // DUR] = 1.0 / DUR

    in_maps = []
    for core in range(N_CORES):
        wslab = np.empty((PAIRS_PER_CORE, K33, DIM, DIM), np.float32)
        xisl = np.zeros((PAIRS_PER_CORE, K33, DIM, K33), np.float32)
        bias_t = np.empty((PAIRS_PER_CORE, K33, DIM), np.float32)
        lnw_t = np.empty((PAIRS_PER_CORE, K33, DIM), np.float32)
        lnb_t = np.empty((PAIRS_PER_CORE, K33, DIM), np.float32)
        segmask = np.zeros((PAIRS_PER_CORE, K33, 2), np.float32)
        ninv = np.empty((PAIRS_PER_CORE, 2, 1), np.float32)
        tmap = np.empty(PAIRS_PER_CORE * K33, np.int64)

        for Pl in range(PAIRS_PER_CORE):
            p = PAIRS_PER_CORE * core + Pl
            b, bp, k, kp, L, Lp = _pair_info(p)

            # weight slab: taps [0,k) from branch b, taps [k,33) from b'
            wslab[Pl, :k] = conv_w[b, :, :, :k].transpose(2, 1, 0)
            wslab[Pl, k:] = conv_w[bp, :, :, :kp].transpose(2, 1, 0)

            # im2col: cols [0,L) use branch-b taps, cols [L,33) branch-b'
            for t in range(k):
                xisl[Pl, t, :, 0:L] = xt[:, t:t + L]
            for tl in range(kp):
                xisl[Pl, k + tl, :, L:K33] = xt[:, tl:tl + Lp]

            bias_t[Pl, 0:L] = conv_b[b][None, :]
            bias_t[Pl, L:K33] = conv_b[bp][None, :]
            lnw_t[Pl, 0:L] = ln_w[b, :, :L].T
            lnw_t[Pl, L:K33] = ln_w[bp, :, :Lp].T
            lnb_t[Pl, 0:L] = ln_b[b, :, :L].T
            lnb_t[Pl, L:K33] = ln_b[bp, :, :Lp].T
            segmask[Pl, 0:L, 0] = 1.0
            segmask[Pl, L:K33, 1] = 1.0
            ninv[Pl, 0, 0] = 1.0 / (DIM * L)
            ninv[Pl, 1, 0] = 1.0 / (DIM * Lp)
            tmap[Pl * K33:Pl * K33 + L] = _branch_offset(b) + np.arange(L)
            tmap[Pl * K33 + L:(Pl + 1) * K33] = _branch_offset(bp) + np.arange(Lp)

        wv_cols = np.zeros((PAIRS_PER_CORE, K33, S1_PAD), np.float32)
        wv_cols[:, :, :T_TOTAL] = Wv[:, tmap].T.reshape(PAIRS_PER_CORE, K33,
                                                        T_TOTAL)

        in_maps.append({
            "wslab": np.ascontiguousarray(
                wslab.reshape(PAIRS_PER_CORE, CTRACT, DIM)
                     .reshape(PAIRS_PER_CORE, NCT // W_CHUNK, W_CHUNK, 128, DIM)
                     .transpose(0, 1, 3, 2, 4)
                     .reshape(PAIRS_PER_CORE, NCT // W_CHUNK, 128,
                              W_CHUNK * DIM)),
            "xislab": np.ascontiguousarray(
                xisl.reshape(PAIRS_PER_CORE, CTRACT, K33)
                    .reshape(PAIRS_PER_CORE, NCT, 128, K33)
                    .transpose(0, 2, 1, 3)
                    .reshape(PAIRS_PER_CORE, 128, NCT * K33)),
            "bias_t": bias_t,
            "lnw_t": lnw_t,
            "lnb_t": lnb_t,
            "segmask": segmask,
            "segmask_tr": np.ascontiguousarray(segmask.transpose(0, 2, 1)),
            "ninv": ninv,
            "wv_cols": wv_cols,
            "bv_pad": bv_pad,
            "wout_tr": wout_tr,
            "opb_sel": opb_sel,
            "winmask": winmask,
        })
    return in_maps


def kernel(**inputs):
    global LAST_EXEC_TIME_NS, LAST_TRACE_DIR
    trace = bool(int(os.environ.get("KERNEL_TRACE", "0")))
    if trace:
        _install_ntff_hook()

    if "nc" not in _PROGRAM_CACHE:
        _PROGRAM_CACHE["nc"] = _build_program()
    nc = _PROGRAM_CACHE["nc"]

    in_maps = _host_prepare(inputs)

    kwargs = {}
    if trace:
        import tempfile
        LAST_TRACE_DIR = tempfile.mkdtemp(prefix="phaseformer_trace_")
        kwargs = dict(trace=True, tmpdir=LAST_TRACE_DIR)
    res = run_bass_kernel_spmd(nc, in_maps, list(range(N_CORES)), **kwargs)
    LAST_EXEC_TIME_NS = res.exec_time_ns

    acc = np.zeros((N_W, DIM), np.float64)
    for i in range(N_CORES):
        acc += res.results[i]["out"].astype(np.float64)
    return acc.astype(np.float32).reshape(1, N_W, DIM)
